# revision 16
# baseline (speedup 1.0000x reference)
"""Balanced CE loss kernel for Trainium2 (8 NeuronCores, data parallel).

Math recap of the reference:
  - ce[b,n] = -log_softmax(inputs[b,n,:2])[target[b,n]]
            = softplus((x0-x1) * (2*t-1))           (two-class CE)
  - scores = uniform(key(42), (B,N))  -- a COMPILE-TIME CONSTANT
  - per row: mean of ce over the top-`num_pos`-by-score positives and the
    top-`num_neg`-by-score negatives; valid-count capped by count_pos.
  - loss = mean_b 0.5 * (pos_mean + neg_mean)

Only ~64 data-dependent positions per row matter, all of which lie inside the
first K=512 positions of the (constant) score-descending order with
overwhelming probability (Binomial(512, 1/2) tails; also checked at runtime
with an exact fallback).  So per core the device:
  1. streams its 16-row shard of `target` (the memory-bound bulk) and
     computes per-row positive counts (DVE reduce + PE partition-reduce),
  2. computes ce on the 512-long score-ordered prefix (gathered with
     constant indices), selects the first num_pos positives / num_neg
     negatives via a hardware prefix scan, and reduces to per-row sums.
Host does the final 128-row scalar arithmetic with exact integer math.
"""

import numpy as np
from contextlib import ExitStack

B, N, C = 128, 131072, 2
NCORES = 8
ROWS = B // NCORES  # 16 rows per core
K = 512             # score-order prefix depth per row
RPT = 4             # target rows per DMA tile in the counting phase
F = N // 128        # 1024 free elements per partition per row

_cache = {}


def _perm():
    """[B, K] int64: first K positions of each row in score-descending order.

    Must match jax.lax.top_k tie-breaking on the reference's scores exactly,
    so compute it with jax.lax.top_k on the very same scores (CPU backend;
    threefry PRNG is backend-deterministic).
    """
    if "perm" not in _cache:
        import jax

        cpu = jax.devices("cpu")[0]
        with jax.default_device(cpu):
            scores = jax.random.uniform(jax.random.key(42), (B, N), dtype=jax.numpy.float32)
            _, idx = jax.lax.top_k(scores, K)
        _cache["perm"] = np.asarray(jax.device_get(idx)).astype(np.int64)
    return _cache["perm"]


def _build_nc(num_pos: int, num_neg: int):
    """Compile the single-core Bass program (same NEFF on all 8 cores)."""
    key = ("nc", num_pos, num_neg)
    if key in _cache:
        return _cache[key]

    import concourse.bacc as bacc
    import concourse.bass as bass
    import concourse.mybir as mybir
    import concourse.tile as tile

    dt = mybir.dt
    af = mybir.ActivationFunctionType
    alu = mybir.AluOpType

    nc = bacc.Bacc("TRN2", target_bir_lowering=False, debug=False)

    tgt = nc.dram_tensor("tgt", [ROWS, N], dt.int32, kind="ExternalInput")
    gt = nc.dram_tensor("gt", [ROWS, K], dt.int32, kind="ExternalInput")
    gx0 = nc.dram_tensor("gx0", [ROWS, K], dt.float32, kind="ExternalInput")
    gx1 = nc.dram_tensor("gx1", [ROWS, K], dt.float32, kind="ExternalInput")
    out = nc.dram_tensor("out", [ROWS, 3], dt.float32, kind="ExternalOutput")

    with tile.TileContext(nc) as tc:
        with (
            tc.tile_pool(name="big", bufs=1) as big_pool,
            tc.tile_pool(name="stats", bufs=1) as stats_pool,
            tc.tile_pool(name="small", bufs=1) as small_pool,
            tc.tile_pool(name="psum", bufs=1, space=bass.MemorySpace.PSUM) as psum_pool,
        ):
            # Small prefix DMAs go on the gpsimd (SWDGE) queue so the sync
            # (HWDGE) queue starts streaming the big target DMAs with no
            # delay.  The prefix compute chain sits early in program order
            # so Tile runs it on DVE/ACT while the big DMAs stream; the
            # count reduces are split DVE/ACT so neither extends past the
            # DMA window.
            gtt = small_pool.tile([ROWS, K], dt.int32)
            x0 = small_pool.tile([ROWS, K], dt.float32)
            x1 = small_pool.tile([ROWS, K], dt.float32)
            nc.gpsimd.dma_start(gtt[:], gt.ap())
            nc.gpsimd.dma_start(x0[:], gx0.ap())
            nc.gpsimd.dma_start(x1[:], gx1.ap())

            statsf = stats_pool.tile([128, ROWS], dt.float32)
            ones = stats_pool.tile([128, 1], dt.float32)
            nc.vector.memset(ones[:], 1.0)

            tgt_ap = tgt.ap()
            bigts = []
            for g in range(ROWS // RPT):
                bigt = big_pool.tile([128, RPT, F], dt.int32, tag=f"big{g}")
                src = tgt_ap[g * RPT:(g + 1) * RPT, :].rearrange(
                    "r (p f) -> p r f", p=128
                )
                nc.sync.dma_start(bigt[:], src)
                bigts.append(bigt)

            # ---- phase B: prefix selection + CE on [ROWS, K] ----
            tf = small_pool.tile([ROWS, K], dt.float32)
            nc.vector.tensor_copy(tf[:], gtt[:])  # i32 -> f32

            d = small_pool.tile([ROWS, K], dt.float32)
            nc.vector.tensor_sub(d[:], x0[:], x1[:])
            sgn = small_pool.tile([ROWS, K], dt.float32)
            nc.vector.tensor_scalar(sgn[:], tf[:], 2.0, -1.0, op0=alu.mult, op1=alu.add)
            dd = small_pool.tile([ROWS, K], dt.float32)
            nc.vector.tensor_mul(dd[:], d[:], sgn[:])
            # ce = softplus(dd), computed stably from exp/ln tables:
            # ce = relu(dd) + ln(1 + exp(-|dd|))
            rl = small_pool.tile([ROWS, K], dt.float32)
            nc.scalar.activation(rl[:], dd[:], af.Relu)
            ab = small_pool.tile([ROWS, K], dt.float32)
            nc.scalar.activation(ab[:], dd[:], af.Abs)
            ex = small_pool.tile([ROWS, K], dt.float32)
            nc.scalar.activation(ex[:], ab[:], af.Exp, scale=-1.0)
            ln = small_pool.tile([ROWS, K], dt.float32)
            nc.scalar.activation(ln[:], ex[:], af.Ln, bias=1.0)
            ce = small_pool.tile([ROWS, K], dt.float32)
            nc.vector.tensor_add(ce[:], rl[:], ln[:])

            # cumulative counts along the score order
            zeros = small_pool.tile([ROWS, K], dt.float32)
            nc.vector.memset(zeros[:], 0.0)
            cpos = small_pool.tile([ROWS, K], dt.float32)
            nc.vector.tensor_tensor_scan(
                cpos[:], tf[:], zeros[:], 0.0, op0=alu.add, op1=alu.add
            )
            tn = small_pool.tile([ROWS, K], dt.float32)
            nc.vector.tensor_scalar(tn[:], tf[:], -1.0, 1.0, op0=alu.mult, op1=alu.add)
            cneg = small_pool.tile([ROWS, K], dt.float32)
            nc.vector.tensor_tensor_scan(
                cneg[:], tn[:], zeros[:], 0.0, op0=alu.add, op1=alu.add
            )

            # masks: in-class AND within the first num_{pos,neg} of that class
            maskp = small_pool.tile([ROWS, K], dt.float32)
            nc.vector.scalar_tensor_tensor(
                maskp[:], cpos[:], float(num_pos), tf[:], op0=alu.is_le, op1=alu.mult
            )
            maskn = small_pool.tile([ROWS, K], dt.float32)
            nc.vector.scalar_tensor_tensor(
                maskn[:], cneg[:], float(num_neg), tn[:], op0=alu.is_le, op1=alu.mult
            )

            outsb = small_pool.tile([ROWS, 3], dt.float32)
            junk0 = small_pool.tile([ROWS, K], dt.float32)
            nc.vector.scalar_tensor_tensor(
                junk0[:], ce[:], 1.0, maskp[:],
                op0=alu.mult, op1=alu.mult, accum_out=outsb[:, 1:2],
            )
            junk1 = small_pool.tile([ROWS, K], dt.float32)
            nc.vector.scalar_tensor_tensor(
                junk1[:], ce[:], 1.0, maskn[:],
                op0=alu.mult, op1=alu.mult, accum_out=outsb[:, 2:3],
            )

            # ---- phase A compute: per-row positive counts ----
            # Row sums split across DVE (reduce_sum) and ACT (activation
            # accumulate) so neither engine extends past the DMA stream.
            for g in range(ROWS // RPT):
                for r in range(RPT):
                    col = statsf[:, g * RPT + r: g * RPT + r + 1]
                    if r % 2 == 0:
                        nc.vector.reduce_sum(
                            out=col, in_=bigts[g][:, r, :],
                            axis=mybir.AxisListType.X,
                        )
                    else:
                        scratch = big_pool.tile(
                            [128, F], dt.float32, tag=f"scratch{g % 2}"
                        )
                        nc.scalar.activation(
                            scratch[:], bigts[g][:, r, :], af.Copy, accum_out=col
                        )

            cnt_psum = psum_pool.tile([ROWS, 1], dt.float32)
            # counts[b] = sum_p statsf[p, b]
            nc.tensor.matmul(cnt_psum[:], statsf[:], ones[:], start=True, stop=True)
            nc.scalar.copy(outsb[:, 0:1], cnt_psum[:])

            nc.sync.dma_start(out.ap(), outsb[:])

    nc.compile()
    _cache[key] = nc
    return nc


def _host_exact(inputs, target, num_pos, num_neg):
    """Exact replication of the reference (jax on CPU). Safety fallback only."""
    import jax
    import jax.numpy as jnp

    cpu = jax.devices("cpu")[0]
    with jax.default_device(cpu):
        inputs = jnp.asarray(inputs)
        target = jnp.asarray(target)
        scores = jax.random.uniform(jax.random.key(42), (B, N))
        is_pos = target == 1
        is_neg = target == 0
        count_pos = is_pos.sum(axis=-1)
        min_pos = jnp.minimum(count_pos, num_pos)
        min_neg = jnp.minimum((count_pos * num_neg) // num_pos, num_neg)
        logp = jax.nn.log_softmax(inputs, axis=-1)
        ce = -jnp.take_along_axis(logp, target[..., None], axis=-1)[..., 0]

        def sampled_mean(mask, k, min_k):
            s = jnp.where(mask, scores, -jnp.inf)
            _, idx = jax.lax.top_k(s, k)
            sel = jnp.take_along_axis(ce, idx, axis=-1)
            valid = jnp.arange(k)[None, :] < min_k[:, None]
            return jnp.where(valid, sel, 0.0).sum(axis=-1) / jnp.maximum(min_k, 1)

        pos_loss = sampled_mean(is_pos, num_pos, min_pos)
        neg_loss = sampled_mean(is_neg, num_neg, min_neg)
        res = ((pos_loss + neg_loss) * 0.5).mean()
    return np.asarray(jax.device_get(res)).astype(np.float32)


def kernel(**inputs) -> np.ndarray:
    from concourse.bass_utils import run_bass_kernel_spmd

    x = np.ascontiguousarray(np.asarray(inputs["inputs"], dtype=np.float32))
    target = np.ascontiguousarray(np.asarray(inputs["target"], dtype=np.int32))
    num_pos = int(np.asarray(inputs["num_pos"]))
    num_neg = int(np.asarray(inputs["num_neg"]))

    if num_pos <= 0 or num_pos > K or num_neg < 0 or num_neg > K:
        # degenerate configs the device program doesn't cover
        return _host_exact(x, target, num_pos, num_neg)

    perm = _perm()
    gt = np.ascontiguousarray(np.take_along_axis(target, perm, axis=1))
    gx0 = np.ascontiguousarray(np.take_along_axis(x[:, :, 0], perm, axis=1))
    gx1 = np.ascontiguousarray(np.take_along_axis(x[:, :, 1], perm, axis=1))

    nc = _build_nc(num_pos, num_neg)
    core_ids = list(range(NCORES))
    in_maps = [
        {
            "tgt": target[c * ROWS:(c + 1) * ROWS],
            "gt": gt[c * ROWS:(c + 1) * ROWS],
            "gx0": gx0[c * ROWS:(c + 1) * ROWS],
            "gx1": gx1[c * ROWS:(c + 1) * ROWS],
        }
        for c in core_ids
    ]
    res = run_bass_kernel_spmd(nc, in_maps, core_ids, trace=_cache.get("trace", False))
    _cache["last_res"] = res
    outs = np.concatenate([res.results[c]["out"] for c in core_ids], axis=0)  # [B,3]

    count = np.rint(outs[:, 0]).astype(np.int64)
    s_pos = outs[:, 1].astype(np.float32)
    s_neg = outs[:, 2].astype(np.float32)

    min_pos = np.minimum(count, num_pos)                          # exact int
    min_neg = np.minimum((count * num_neg) // num_pos, num_neg)   # exact int

    # Guard: the K-prefix must contain every selected sample; count_neg must
    # cover min_neg (else reference semantics touch -inf slots).  Never fires
    # for the target data (binomial tails ~1e-80); fallback stays exact.
    prefix_pos = gt.sum(axis=1, dtype=np.int64)
    prefix_neg = K - prefix_pos
    count_neg = N - count
    if (
        (prefix_pos < min_pos).any()
        or (prefix_neg < min_neg).any()
        or (count_neg < min_neg).any()
    ):
        return _host_exact(x, target, num_pos, num_neg)

    pos_loss = s_pos / np.maximum(min_pos, 1).astype(np.float32)
    neg_loss = s_neg / np.maximum(min_neg, 1).astype(np.float32)
    loss = np.float32(0.5) * (pos_loss + neg_loss)
    return np.asarray(loss.mean(), dtype=np.float32)


# revision 21
# speedup vs baseline: 1.0413x; 1.0413x over previous
"""Balanced CE loss kernel for Trainium2 (8 NeuronCores, data parallel).

Math recap of the reference:
  - ce[b,n] = -log_softmax(inputs[b,n,:2])[target[b,n]]
            = softplus((x0-x1) * (2*t-1))           (two-class CE)
  - scores = uniform(key(42), (B,N))  -- a COMPILE-TIME CONSTANT
  - per row: mean of ce over the top-`num_pos`-by-score positives and the
    top-`num_neg`-by-score negatives; valid-count capped by count_pos.
  - loss = mean_b 0.5 * (pos_mean + neg_mean)

Only ~64 data-dependent positions per row matter, all of which lie inside the
first K=512 positions of the (constant) score-descending order with
overwhelming probability (Binomial(512, 1/2) tails; also checked at runtime
with an exact fallback).  So per core the device:
  1. streams its 16-row shard of `target` (the memory-bound bulk) and
     computes per-row positive counts (DVE reduce + PE partition-reduce),
  2. computes ce on the 512-long score-ordered prefix (gathered with
     constant indices), selects the first num_pos positives / num_neg
     negatives via a hardware prefix scan, and reduces to per-row sums.
Host does the final 128-row scalar arithmetic with exact integer math.
"""

import numpy as np
from contextlib import ExitStack

B, N, C = 128, 131072, 2
NCORES = 8
ROWS = B // NCORES  # 16 rows per core
K = 512             # score-order prefix depth per row
RPT = 4             # target rows per DMA tile in the counting phase
F = N // 128        # 1024 free elements per partition per row

_cache = {}


def _perm():
    """[B, K] int64: first K positions of each row in score-descending order.

    Must match jax.lax.top_k tie-breaking on the reference's scores exactly,
    so compute it with jax.lax.top_k on the very same scores (CPU backend;
    threefry PRNG is backend-deterministic).
    """
    if "perm" not in _cache:
        import jax

        cpu = jax.devices("cpu")[0]
        with jax.default_device(cpu):
            scores = jax.random.uniform(jax.random.key(42), (B, N), dtype=jax.numpy.float32)
            _, idx = jax.lax.top_k(scores, K)
        _cache["perm"] = np.asarray(jax.device_get(idx)).astype(np.int64)
    return _cache["perm"]


def _build_nc(num_pos: int, num_neg: int):
    """Compile the single-core Bass program (same NEFF on all 8 cores)."""
    key = ("nc", num_pos, num_neg)
    if key in _cache:
        return _cache[key]

    import concourse.bacc as bacc
    import concourse.bass as bass
    import concourse.mybir as mybir
    import concourse.tile as tile

    dt = mybir.dt
    af = mybir.ActivationFunctionType
    alu = mybir.AluOpType

    nc = bacc.Bacc("TRN2", target_bir_lowering=False, debug=False)

    tgt = nc.dram_tensor("tgt", [ROWS, N], dt.int32, kind="ExternalInput")
    gt = nc.dram_tensor("gt", [ROWS, K], dt.int32, kind="ExternalInput")
    gx0 = nc.dram_tensor("gx0", [ROWS, K], dt.float32, kind="ExternalInput")
    gx1 = nc.dram_tensor("gx1", [ROWS, K], dt.float32, kind="ExternalInput")
    out = nc.dram_tensor("out", [ROWS, 3], dt.float32, kind="ExternalOutput")

    with tile.TileContext(nc) as tc:
        with (
            tc.tile_pool(name="big", bufs=1) as big_pool,
            tc.tile_pool(name="stats", bufs=1) as stats_pool,
            tc.tile_pool(name="small", bufs=1) as small_pool,
            tc.tile_pool(name="psum", bufs=1, space=bass.MemorySpace.PSUM) as psum_pool,
        ):
            # Small prefix DMAs go on the gpsimd (SWDGE) queue so the sync
            # (HWDGE) queue starts streaming the big target DMAs with no
            # delay.  The prefix compute chain sits early in program order
            # so Tile runs it on DVE/ACT while the big DMAs stream; the
            # count reduces are split DVE/ACT so neither extends past the
            # DMA window.
            gtt = small_pool.tile([ROWS, K], dt.int32)
            x0 = small_pool.tile([ROWS, K], dt.float32)
            x1 = small_pool.tile([ROWS, K], dt.float32)
            nc.gpsimd.dma_start(gtt[:], gt.ap())
            nc.gpsimd.dma_start(x0[:], gx0.ap())
            nc.gpsimd.dma_start(x1[:], gx1.ap())

            statsf = stats_pool.tile([128, ROWS], dt.float32)
            ones = stats_pool.tile([128, 1], dt.float32)
            nc.vector.memset(ones[:], 1.0)

            # 2MB tiles keep the HWDGE queue saturated; the last two groups
            # are half-size so the final row-sums tail past the DMA stream
            # is minimal.
            GROUPS = [4, 4, 4, 2, 2]
            assert sum(GROUPS) == ROWS
            tgt_ap = tgt.ap()
            bigts = []
            row0 = 0
            for g, sz in enumerate(GROUPS):
                bigt = big_pool.tile([128, sz, F], dt.int32, tag=f"big{g}")
                src = tgt_ap[row0:row0 + sz, :].rearrange("r (p f) -> p r f", p=128)
                nc.sync.dma_start(bigt[:], src)
                bigts.append(bigt)
                row0 += sz

            # ---- phase B: prefix selection + CE on [ROWS, K] ----
            tf = small_pool.tile([ROWS, K], dt.float32)
            nc.vector.tensor_copy(tf[:], gtt[:])  # i32 -> f32

            d = small_pool.tile([ROWS, K], dt.float32)
            nc.vector.tensor_sub(d[:], x0[:], x1[:])
            sgn = small_pool.tile([ROWS, K], dt.float32)
            nc.vector.tensor_scalar(sgn[:], tf[:], 2.0, -1.0, op0=alu.mult, op1=alu.add)
            dd = small_pool.tile([ROWS, K], dt.float32)
            nc.vector.tensor_mul(dd[:], d[:], sgn[:])
            # ce = softplus(dd) = relu(dd) + ln(1 + exp(-|dd|))
            # relu/abs on DVE; only exp/ln need the ACT tables
            rl = small_pool.tile([ROWS, K], dt.float32)
            nc.vector.tensor_scalar_max(rl[:], dd[:], 0.0)
            ab = small_pool.tile([ROWS, K], dt.float32)
            nc.vector.scalar_tensor_tensor(
                ab[:], rl[:], 2.0, dd[:], op0=alu.mult, op1=alu.subtract
            )
            ex = small_pool.tile([ROWS, K], dt.float32)
            nc.scalar.activation(ex[:], ab[:], af.Exp, scale=-1.0)
            ln = small_pool.tile([ROWS, K], dt.float32)
            nc.scalar.activation(ln[:], ex[:], af.Ln, bias=1.0)
            ce = small_pool.tile([ROWS, K], dt.float32)
            nc.vector.tensor_add(ce[:], rl[:], ln[:])

            # cumulative counts along the score order
            zeros = small_pool.tile([ROWS, K], dt.float32)
            nc.vector.memset(zeros[:], 0.0)
            cpos = small_pool.tile([ROWS, K], dt.float32)
            nc.vector.tensor_tensor_scan(
                cpos[:], tf[:], zeros[:], 0.0, op0=alu.add, op1=alu.add
            )
            tn = small_pool.tile([ROWS, K], dt.float32)
            nc.vector.tensor_scalar(tn[:], tf[:], -1.0, 1.0, op0=alu.mult, op1=alu.add)
            cneg = small_pool.tile([ROWS, K], dt.float32)
            nc.vector.tensor_tensor_scan(
                cneg[:], tn[:], zeros[:], 0.0, op0=alu.add, op1=alu.add
            )

            # masks: in-class AND within the first num_{pos,neg} of that class
            maskp = small_pool.tile([ROWS, K], dt.float32)
            nc.vector.scalar_tensor_tensor(
                maskp[:], cpos[:], float(num_pos), tf[:], op0=alu.is_le, op1=alu.mult
            )
            maskn = small_pool.tile([ROWS, K], dt.float32)
            nc.vector.scalar_tensor_tensor(
                maskn[:], cneg[:], float(num_neg), tn[:], op0=alu.is_le, op1=alu.mult
            )

            outsb = small_pool.tile([ROWS, 3], dt.float32)
            junk0 = small_pool.tile([ROWS, K], dt.float32)
            nc.vector.scalar_tensor_tensor(
                junk0[:], ce[:], 1.0, maskp[:],
                op0=alu.mult, op1=alu.mult, accum_out=outsb[:, 1:2],
            )
            junk1 = small_pool.tile([ROWS, K], dt.float32)
            nc.vector.scalar_tensor_tensor(
                junk1[:], ce[:], 1.0, maskn[:],
                op0=alu.mult, op1=alu.mult, accum_out=outsb[:, 2:3],
            )

            # ---- phase A compute: per-row positive counts ----
            # Row sums split across DVE (reduce_sum) and ACT (activation
            # accumulate) so neither engine extends past the DMA stream.
            row0 = 0
            for g, sz in enumerate(GROUPS):
                for r in range(sz):
                    col = statsf[:, row0 + r: row0 + r + 1]
                    if r % 2 == 0:
                        nc.vector.reduce_sum(
                            out=col, in_=bigts[g][:, r, :],
                            axis=mybir.AxisListType.X,
                        )
                    else:
                        scratch = big_pool.tile(
                            [128, F], dt.float32, tag=f"scratch{g % 2}"
                        )
                        nc.scalar.activation(
                            scratch[:], bigts[g][:, r, :], af.Copy, accum_out=col
                        )
                row0 += sz

            cnt_psum = psum_pool.tile([ROWS, 1], dt.float32)
            # counts[b] = sum_p statsf[p, b]
            nc.tensor.matmul(cnt_psum[:], statsf[:], ones[:], start=True, stop=True)
            nc.scalar.copy(outsb[:, 0:1], cnt_psum[:])

            nc.sync.dma_start(out.ap(), outsb[:])

    nc.compile()
    _cache[key] = nc
    return nc


def _host_exact(inputs, target, num_pos, num_neg):
    """Exact replication of the reference (jax on CPU). Safety fallback only."""
    import jax
    import jax.numpy as jnp

    cpu = jax.devices("cpu")[0]
    with jax.default_device(cpu):
        inputs = jnp.asarray(inputs)
        target = jnp.asarray(target)
        scores = jax.random.uniform(jax.random.key(42), (B, N))
        is_pos = target == 1
        is_neg = target == 0
        count_pos = is_pos.sum(axis=-1)
        min_pos = jnp.minimum(count_pos, num_pos)
        min_neg = jnp.minimum((count_pos * num_neg) // num_pos, num_neg)
        logp = jax.nn.log_softmax(inputs, axis=-1)
        ce = -jnp.take_along_axis(logp, target[..., None], axis=-1)[..., 0]

        def sampled_mean(mask, k, min_k):
            s = jnp.where(mask, scores, -jnp.inf)
            _, idx = jax.lax.top_k(s, k)
            sel = jnp.take_along_axis(ce, idx, axis=-1)
            valid = jnp.arange(k)[None, :] < min_k[:, None]
            return jnp.where(valid, sel, 0.0).sum(axis=-1) / jnp.maximum(min_k, 1)

        pos_loss = sampled_mean(is_pos, num_pos, min_pos)
        neg_loss = sampled_mean(is_neg, num_neg, min_neg)
        res = ((pos_loss + neg_loss) * 0.5).mean()
    return np.asarray(jax.device_get(res)).astype(np.float32)


def kernel(**inputs) -> np.ndarray:
    from concourse.bass_utils import run_bass_kernel_spmd

    x = np.ascontiguousarray(np.asarray(inputs["inputs"], dtype=np.float32))
    target = np.ascontiguousarray(np.asarray(inputs["target"], dtype=np.int32))
    num_pos = int(np.asarray(inputs["num_pos"]))
    num_neg = int(np.asarray(inputs["num_neg"]))

    if num_pos <= 0 or num_pos > K or num_neg < 0 or num_neg > K:
        # degenerate configs the device program doesn't cover
        return _host_exact(x, target, num_pos, num_neg)

    perm = _perm()
    gt = np.ascontiguousarray(np.take_along_axis(target, perm, axis=1))
    gx0 = np.ascontiguousarray(np.take_along_axis(x[:, :, 0], perm, axis=1))
    gx1 = np.ascontiguousarray(np.take_along_axis(x[:, :, 1], perm, axis=1))

    nc = _build_nc(num_pos, num_neg)
    core_ids = list(range(NCORES))
    in_maps = [
        {
            "tgt": target[c * ROWS:(c + 1) * ROWS],
            "gt": gt[c * ROWS:(c + 1) * ROWS],
            "gx0": gx0[c * ROWS:(c + 1) * ROWS],
            "gx1": gx1[c * ROWS:(c + 1) * ROWS],
        }
        for c in core_ids
    ]
    res = run_bass_kernel_spmd(nc, in_maps, core_ids, trace=_cache.get("trace", False))
    _cache["last_res"] = res
    outs = np.concatenate([res.results[c]["out"] for c in core_ids], axis=0)  # [B,3]

    count = np.rint(outs[:, 0]).astype(np.int64)
    s_pos = outs[:, 1].astype(np.float32)
    s_neg = outs[:, 2].astype(np.float32)

    min_pos = np.minimum(count, num_pos)                          # exact int
    min_neg = np.minimum((count * num_neg) // num_pos, num_neg)   # exact int

    # Guard: the K-prefix must contain every selected sample; count_neg must
    # cover min_neg (else reference semantics touch -inf slots).  Never fires
    # for the target data (binomial tails ~1e-80); fallback stays exact.
    prefix_pos = gt.sum(axis=1, dtype=np.int64)
    prefix_neg = K - prefix_pos
    count_neg = N - count
    if (
        (prefix_pos < min_pos).any()
        or (prefix_neg < min_neg).any()
        or (count_neg < min_neg).any()
    ):
        return _host_exact(x, target, num_pos, num_neg)

    pos_loss = s_pos / np.maximum(min_pos, 1).astype(np.float32)
    neg_loss = s_neg / np.maximum(min_neg, 1).astype(np.float32)
    loss = np.float32(0.5) * (pos_loss + neg_loss)
    return np.asarray(loss.mean(), dtype=np.float32)


# revision 24
# speedup vs baseline: 1.0533x; 1.0116x over previous
"""Balanced CE loss kernel for Trainium2 (8 NeuronCores, data parallel).

Math recap of the reference:
  - ce[b,n] = -log_softmax(inputs[b,n,:2])[target[b,n]]
            = softplus((x0-x1) * (2*t-1))           (two-class CE)
  - scores = uniform(key(42), (B,N))  -- a COMPILE-TIME CONSTANT
  - per row: mean of ce over the top-`num_pos`-by-score positives and the
    top-`num_neg`-by-score negatives; valid-count capped by count_pos.
  - loss = mean_b 0.5 * (pos_mean + neg_mean)

Only ~64 data-dependent positions per row matter, all of which lie inside the
first K=512 positions of the (constant) score-descending order with
overwhelming probability (Binomial(512, 1/2) tails; also checked at runtime
with an exact fallback).  So per core the device:
  1. streams its 16-row shard of `target` (the memory-bound bulk) and
     computes per-row positive counts (DVE reduce + PE partition-reduce),
  2. computes ce on the 512-long score-ordered prefix (gathered with
     constant indices), selects the first num_pos positives / num_neg
     negatives via a hardware prefix scan, and reduces to per-row sums.
Host does the final 128-row scalar arithmetic with exact integer math.
"""

import numpy as np
from contextlib import ExitStack

B, N, C = 128, 131072, 2
NCORES = 8
ROWS = B // NCORES  # 16 rows per core
K = 512             # score-order prefix depth per row
RPT = 4             # target rows per DMA tile in the counting phase
F = N // 128        # 1024 free elements per partition per row

_cache = {}


def _perm():
    """[B, K] int64: first K positions of each row in score-descending order.

    Must match jax.lax.top_k tie-breaking on the reference's scores exactly,
    so compute it with jax.lax.top_k on the very same scores (CPU backend;
    threefry PRNG is backend-deterministic).
    """
    if "perm" not in _cache:
        import jax

        cpu = jax.devices("cpu")[0]
        with jax.default_device(cpu):
            scores = jax.random.uniform(jax.random.key(42), (B, N), dtype=jax.numpy.float32)
            _, idx = jax.lax.top_k(scores, K)
        _cache["perm"] = np.asarray(jax.device_get(idx)).astype(np.int64)
    return _cache["perm"]


def _build_nc(num_pos: int, num_neg: int):
    """Compile the single-core Bass program (same NEFF on all 8 cores)."""
    key = ("nc", num_pos, num_neg)
    if key in _cache:
        return _cache[key]

    import concourse.bacc as bacc
    import concourse.bass as bass
    import concourse.mybir as mybir
    import concourse.tile as tile

    dt = mybir.dt
    af = mybir.ActivationFunctionType
    alu = mybir.AluOpType

    nc = bacc.Bacc("TRN2", target_bir_lowering=False, debug=False)

    tgt = nc.dram_tensor("tgt", [ROWS, N], dt.int32, kind="ExternalInput")
    gt = nc.dram_tensor("gt", [ROWS, K], dt.int32, kind="ExternalInput")
    gx0 = nc.dram_tensor("gx0", [ROWS, K], dt.float32, kind="ExternalInput")
    gx1 = nc.dram_tensor("gx1", [ROWS, K], dt.float32, kind="ExternalInput")
    out = nc.dram_tensor("out", [ROWS, 3], dt.float32, kind="ExternalOutput")

    with tile.TileContext(nc) as tc:
        with (
            tc.tile_pool(name="big", bufs=1) as big_pool,
            tc.tile_pool(name="stats", bufs=1) as stats_pool,
            tc.tile_pool(name="small", bufs=1) as small_pool,
            tc.tile_pool(name="psum", bufs=1, space=bass.MemorySpace.PSUM) as psum_pool,
        ):
            # Small prefix DMAs go on the gpsimd (SWDGE) queue so the sync
            # (HWDGE) queue starts streaming the big target DMAs with no
            # delay.  The prefix compute chain sits early in program order
            # so Tile runs it on DVE/ACT while the big DMAs stream; the
            # count reduces are split DVE/ACT so neither extends past the
            # DMA window.
            gtt = small_pool.tile([ROWS, K], dt.int32)
            x0 = small_pool.tile([ROWS, K], dt.float32)
            x1 = small_pool.tile([ROWS, K], dt.float32)
            nc.gpsimd.dma_start(gtt[:], gt.ap())
            nc.gpsimd.dma_start(x0[:], gx0.ap())
            nc.gpsimd.dma_start(x1[:], gx1.ap())

            statsf = stats_pool.tile([128, ROWS], dt.float32)
            ones = stats_pool.tile([128, 1], dt.float32)
            nc.vector.memset(ones[:], 1.0)

            # 2MB tiles keep the HWDGE queue saturated; the last two groups
            # are half-size so the final row-sums tail past the DMA stream
            # is minimal.
            GROUPS = [4, 4, 4, 2, 2]
            assert sum(GROUPS) == ROWS
            tgt_ap = tgt.ap()
            bigts = []
            row0 = 0
            for g, sz in enumerate(GROUPS):
                bigt = big_pool.tile([128, sz, F], dt.int32, tag=f"big{g}")
                src = tgt_ap[row0:row0 + sz, :].rearrange("r (p f) -> p r f", p=128)
                nc.sync.dma_start(bigt[:], src)
                bigts.append(bigt)
                row0 += sz

            # ---- phase B: prefix selection + CE on [ROWS, K] ----
            tf = small_pool.tile([ROWS, K], dt.float32)
            nc.vector.tensor_copy(tf[:], gtt[:])  # i32 -> f32

            d = small_pool.tile([ROWS, K], dt.float32)
            nc.vector.tensor_sub(d[:], x0[:], x1[:])
            sgn = small_pool.tile([ROWS, K], dt.float32)
            nc.vector.tensor_scalar(sgn[:], tf[:], 2.0, -1.0, op0=alu.mult, op1=alu.add)
            dd = small_pool.tile([ROWS, K], dt.float32)
            nc.vector.tensor_mul(dd[:], d[:], sgn[:])
            # ce = softplus(dd) = relu(dd) + ln(1 + exp(-|dd|))
            # relu/abs on DVE; only exp/ln need the ACT tables
            rl = small_pool.tile([ROWS, K], dt.float32)
            nc.vector.tensor_scalar_max(rl[:], dd[:], 0.0)
            ab = small_pool.tile([ROWS, K], dt.float32)
            nc.vector.scalar_tensor_tensor(
                ab[:], rl[:], 2.0, dd[:], op0=alu.mult, op1=alu.subtract
            )
            ex = small_pool.tile([ROWS, K], dt.float32)
            nc.scalar.activation(ex[:], ab[:], af.Exp, scale=-1.0)
            ln = small_pool.tile([ROWS, K], dt.float32)
            nc.scalar.activation(ln[:], ex[:], af.Ln, bias=1.0)
            ce = small_pool.tile([ROWS, K], dt.float32)
            nc.vector.tensor_add(ce[:], rl[:], ln[:])

            # cumulative counts along the score order (gpsimd: it is idle,
            # and this chain only needs tf -- fully parallel to the DVE ce
            # chain)
            zeros = small_pool.tile([ROWS, K], dt.float32)
            nc.gpsimd.memset(zeros[:], 0.0)
            cpos = small_pool.tile([ROWS, K], dt.float32)
            nc.vector.tensor_tensor_scan(
                cpos[:], tf[:], zeros[:], 0.0, op0=alu.add, op1=alu.add
            )
            tn = small_pool.tile([ROWS, K], dt.float32)
            nc.vector.tensor_scalar(tn[:], tf[:], -1.0, 1.0, op0=alu.mult, op1=alu.add)
            cneg = small_pool.tile([ROWS, K], dt.float32)
            nc.vector.tensor_tensor_scan(
                cneg[:], tn[:], zeros[:], 0.0, op0=alu.add, op1=alu.add
            )

            # masks: in-class AND within the first num_{pos,neg} of that class
            maskp = small_pool.tile([ROWS, K], dt.float32)
            nc.vector.scalar_tensor_tensor(
                maskp[:], cpos[:], float(num_pos), tf[:], op0=alu.is_le, op1=alu.mult
            )
            maskn = small_pool.tile([ROWS, K], dt.float32)
            nc.vector.scalar_tensor_tensor(
                maskn[:], cneg[:], float(num_neg), tn[:], op0=alu.is_le, op1=alu.mult
            )

            outsb = small_pool.tile([ROWS, 3], dt.float32)
            junk0 = small_pool.tile([ROWS, K], dt.float32)
            nc.vector.scalar_tensor_tensor(
                junk0[:], ce[:], 1.0, maskp[:],
                op0=alu.mult, op1=alu.mult, accum_out=outsb[:, 1:2],
            )
            junk1 = small_pool.tile([ROWS, K], dt.float32)
            nc.vector.scalar_tensor_tensor(
                junk1[:], ce[:], 1.0, maskn[:],
                op0=alu.mult, op1=alu.mult, accum_out=outsb[:, 2:3],
            )

            # ---- phase A compute: per-row positive counts ----
            # Row sums split across DVE (reduce_sum) and ACT (activation
            # accumulate) so neither engine extends past the DMA stream.
            row0 = 0
            for g, sz in enumerate(GROUPS):
                for r in range(sz):
                    col = statsf[:, row0 + r: row0 + r + 1]
                    if r % 2 == 0:
                        nc.vector.reduce_sum(
                            out=col, in_=bigts[g][:, r, :],
                            axis=mybir.AxisListType.X,
                        )
                    else:
                        scratch = big_pool.tile(
                            [128, F], dt.float32, tag=f"scratch{g % 2}"
                        )
                        nc.scalar.activation(
                            scratch[:], bigts[g][:, r, :], af.Copy, accum_out=col
                        )
                row0 += sz

            cnt_psum = psum_pool.tile([ROWS, 1], dt.float32)
            # counts[b] = sum_p statsf[p, b]
            nc.tensor.matmul(cnt_psum[:], statsf[:], ones[:], start=True, stop=True)
            nc.scalar.copy(outsb[:, 0:1], cnt_psum[:])

            nc.sync.dma_start(out.ap(), outsb[:])

    nc.compile()
    _cache[key] = nc
    return nc


def _host_exact(inputs, target, num_pos, num_neg):
    """Exact replication of the reference (jax on CPU). Safety fallback only."""
    import jax
    import jax.numpy as jnp

    cpu = jax.devices("cpu")[0]
    with jax.default_device(cpu):
        inputs = jnp.asarray(inputs)
        target = jnp.asarray(target)
        scores = jax.random.uniform(jax.random.key(42), (B, N))
        is_pos = target == 1
        is_neg = target == 0
        count_pos = is_pos.sum(axis=-1)
        min_pos = jnp.minimum(count_pos, num_pos)
        min_neg = jnp.minimum((count_pos * num_neg) // num_pos, num_neg)
        logp = jax.nn.log_softmax(inputs, axis=-1)
        ce = -jnp.take_along_axis(logp, target[..., None], axis=-1)[..., 0]

        def sampled_mean(mask, k, min_k):
            s = jnp.where(mask, scores, -jnp.inf)
            _, idx = jax.lax.top_k(s, k)
            sel = jnp.take_along_axis(ce, idx, axis=-1)
            valid = jnp.arange(k)[None, :] < min_k[:, None]
            return jnp.where(valid, sel, 0.0).sum(axis=-1) / jnp.maximum(min_k, 1)

        pos_loss = sampled_mean(is_pos, num_pos, min_pos)
        neg_loss = sampled_mean(is_neg, num_neg, min_neg)
        res = ((pos_loss + neg_loss) * 0.5).mean()
    return np.asarray(jax.device_get(res)).astype(np.float32)


def kernel(**inputs) -> np.ndarray:
    from concourse.bass_utils import run_bass_kernel_spmd

    x = np.ascontiguousarray(np.asarray(inputs["inputs"], dtype=np.float32))
    target = np.ascontiguousarray(np.asarray(inputs["target"], dtype=np.int32))
    num_pos = int(np.asarray(inputs["num_pos"]))
    num_neg = int(np.asarray(inputs["num_neg"]))

    if num_pos <= 0 or num_pos > K or num_neg < 0 or num_neg > K:
        # degenerate configs the device program doesn't cover
        return _host_exact(x, target, num_pos, num_neg)

    perm = _perm()
    gt = np.ascontiguousarray(np.take_along_axis(target, perm, axis=1))
    gx0 = np.ascontiguousarray(np.take_along_axis(x[:, :, 0], perm, axis=1))
    gx1 = np.ascontiguousarray(np.take_along_axis(x[:, :, 1], perm, axis=1))

    nc = _build_nc(num_pos, num_neg)
    core_ids = list(range(NCORES))
    in_maps = [
        {
            "tgt": target[c * ROWS:(c + 1) * ROWS],
            "gt": gt[c * ROWS:(c + 1) * ROWS],
            "gx0": gx0[c * ROWS:(c + 1) * ROWS],
            "gx1": gx1[c * ROWS:(c + 1) * ROWS],
        }
        for c in core_ids
    ]
    res = run_bass_kernel_spmd(nc, in_maps, core_ids, trace=_cache.get("trace", False))
    _cache["last_res"] = res
    outs = np.concatenate([res.results[c]["out"] for c in core_ids], axis=0)  # [B,3]

    count = np.rint(outs[:, 0]).astype(np.int64)
    s_pos = outs[:, 1].astype(np.float32)
    s_neg = outs[:, 2].astype(np.float32)

    min_pos = np.minimum(count, num_pos)                          # exact int
    min_neg = np.minimum((count * num_neg) // num_pos, num_neg)   # exact int

    # Guard: the K-prefix must contain every selected sample; count_neg must
    # cover min_neg (else reference semantics touch -inf slots).  Never fires
    # for the target data (binomial tails ~1e-80); fallback stays exact.
    prefix_pos = gt.sum(axis=1, dtype=np.int64)
    prefix_neg = K - prefix_pos
    count_neg = N - count
    if (
        (prefix_pos < min_pos).any()
        or (prefix_neg < min_neg).any()
        or (count_neg < min_neg).any()
    ):
        return _host_exact(x, target, num_pos, num_neg)

    pos_loss = s_pos / np.maximum(min_pos, 1).astype(np.float32)
    neg_loss = s_neg / np.maximum(min_neg, 1).astype(np.float32)
    loss = np.float32(0.5) * (pos_loss + neg_loss)
    return np.asarray(loss.mean(), dtype=np.float32)


# revision 25
# speedup vs baseline: 1.0906x; 1.0353x over previous
"""Balanced CE loss kernel for Trainium2 (8 NeuronCores, data parallel).

Math recap of the reference:
  - ce[b,n] = -log_softmax(inputs[b,n,:2])[target[b,n]]
            = softplus((x0-x1) * (2*t-1))           (two-class CE)
  - scores = uniform(key(42), (B,N))  -- a COMPILE-TIME CONSTANT
  - per row: mean of ce over the top-`num_pos`-by-score positives and the
    top-`num_neg`-by-score negatives; valid-count capped by count_pos.
  - loss = mean_b 0.5 * (pos_mean + neg_mean)

Only ~64 data-dependent positions per row matter, all of which lie inside the
first K=512 positions of the (constant) score-descending order with
overwhelming probability (Binomial(512, 1/2) tails; also checked at runtime
with an exact fallback).  So per core the device:
  1. streams its 16-row shard of `target` (the memory-bound bulk) and
     computes per-row positive counts (DVE reduce + PE partition-reduce),
  2. computes ce on the 512-long score-ordered prefix (gathered with
     constant indices), selects the first num_pos positives / num_neg
     negatives via a hardware prefix scan, and reduces to per-row sums.
Host does the final 128-row scalar arithmetic with exact integer math.
"""

import numpy as np

B, N, C = 128, 131072, 2
NCORES = 8
ROWS = B // NCORES  # 16 rows per core
K = 512             # score-order prefix depth per row
F = N // 128        # 1024 free elements per partition per row

_cache = {}


def _perm():
    """[B, K] int64: first K positions of each row in score-descending order.

    Must match jax.lax.top_k tie-breaking on the reference's scores exactly,
    so compute it with jax.lax.top_k on the very same scores (CPU backend;
    threefry PRNG is backend-deterministic).
    """
    if "perm" not in _cache:
        import jax

        cpu = jax.devices("cpu")[0]
        with jax.default_device(cpu):
            scores = jax.random.uniform(jax.random.key(42), (B, N), dtype=jax.numpy.float32)
            _, idx = jax.lax.top_k(scores, K)
        _cache["perm"] = np.asarray(jax.device_get(idx)).astype(np.int64)
    return _cache["perm"]


def _build_nc(num_pos: int, num_neg: int):
    """Compile the single-core Bass program (same NEFF on all 8 cores)."""
    key = ("nc", num_pos, num_neg)
    if key in _cache:
        return _cache[key]

    import concourse.bacc as bacc
    import concourse.bass as bass
    import concourse.mybir as mybir
    import concourse.tile as tile

    dt = mybir.dt
    af = mybir.ActivationFunctionType
    alu = mybir.AluOpType

    nc = bacc.Bacc("TRN2", target_bir_lowering=False, debug=False)

    tgt = nc.dram_tensor("tgt", [ROWS, N], dt.int32, kind="ExternalInput")
    gt = nc.dram_tensor("gt", [ROWS, K], dt.int32, kind="ExternalInput")
    gx0 = nc.dram_tensor("gx0", [ROWS, K], dt.float32, kind="ExternalInput")
    gx1 = nc.dram_tensor("gx1", [ROWS, K], dt.float32, kind="ExternalInput")
    out = nc.dram_tensor("out", [ROWS, 3], dt.float32, kind="ExternalOutput")

    with tile.TileContext(nc) as tc:
        with (
            tc.tile_pool(name="big", bufs=1) as big_pool,
            tc.tile_pool(name="stats", bufs=1) as stats_pool,
            tc.tile_pool(name="small", bufs=1) as small_pool,
            tc.tile_pool(name="psum", bufs=1, space=bass.MemorySpace.PSUM) as psum_pool,
        ):
            # Small prefix DMAs go on the gpsimd (SWDGE) queue so the sync
            # (HWDGE) queue starts streaming the big target DMAs with no
            # delay.  The prefix compute chain sits early in program order
            # so Tile runs it on DVE/ACT while the big DMAs stream; the
            # count reduces are split DVE/ACT so neither extends past the
            # DMA window.
            gtt = small_pool.tile([ROWS, K], dt.int32)
            x0 = small_pool.tile([ROWS, K], dt.float32)
            x1 = small_pool.tile([ROWS, K], dt.float32)
            nc.gpsimd.dma_start(gtt[:], gt.ap())
            nc.gpsimd.dma_start(x0[:], gx0.ap())
            nc.gpsimd.dma_start(x1[:], gx1.ap())

            statsf = stats_pool.tile([128, ROWS], dt.float32)
            ones = stats_pool.tile([128, 1], dt.float32)
            nc.vector.memset(ones[:], 1.0)

            # 2MB tiles keep the HWDGE queue saturated; the last two groups
            # are half-size so the final row-sums tail past the DMA stream
            # is minimal.
            GROUPS = [4, 4, 4, 2, 2]
            assert sum(GROUPS) == ROWS
            tgt_ap = tgt.ap()
            bigts = []
            row0 = 0
            for g, sz in enumerate(GROUPS):
                bigt = big_pool.tile([128, sz, F], dt.int32, tag=f"big{g}")
                src = tgt_ap[row0:row0 + sz, :].rearrange("r (p f) -> p r f", p=128)
                nc.sync.dma_start(bigt[:], src)
                bigts.append(bigt)
                row0 += sz

            # ---- phase B: prefix selection + CE on [ROWS, K] ----
            tf = small_pool.tile([ROWS, K], dt.float32)
            nc.vector.tensor_copy(tf[:], gtt[:])  # i32 -> f32

            d = small_pool.tile([ROWS, K], dt.float32)
            nc.vector.tensor_sub(d[:], x0[:], x1[:])
            sgn = small_pool.tile([ROWS, K], dt.float32)
            nc.vector.tensor_scalar(sgn[:], tf[:], 2.0, -1.0, op0=alu.mult, op1=alu.add)
            dd = small_pool.tile([ROWS, K], dt.float32)
            nc.vector.tensor_mul(dd[:], d[:], sgn[:])
            # ce = softplus(dd) = relu(dd) + ln(1 + exp(-|dd|))
            # relu/abs on DVE; only exp/ln need the ACT tables
            rl = small_pool.tile([ROWS, K], dt.float32)
            nc.vector.tensor_scalar_max(rl[:], dd[:], 0.0)
            ab = small_pool.tile([ROWS, K], dt.float32)
            nc.vector.scalar_tensor_tensor(
                ab[:], rl[:], 2.0, dd[:], op0=alu.mult, op1=alu.subtract
            )
            ex = small_pool.tile([ROWS, K], dt.float32)
            nc.scalar.activation(ex[:], ab[:], af.Exp, scale=-1.0)
            ln = small_pool.tile([ROWS, K], dt.float32)
            nc.scalar.activation(ln[:], ex[:], af.Ln, bias=1.0)
            ce = small_pool.tile([ROWS, K], dt.float32)
            nc.vector.tensor_add(ce[:], rl[:], ln[:])

            # cumulative counts along the score order (gpsimd: it is idle,
            # and this chain only needs tf -- fully parallel to the DVE ce
            # chain)
            zeros = small_pool.tile([ROWS, K], dt.float32)
            nc.gpsimd.memset(zeros[:], 0.0)
            cpos = small_pool.tile([ROWS, K], dt.float32)
            nc.vector.tensor_tensor_scan(
                cpos[:], tf[:], zeros[:], 0.0, op0=alu.add, op1=alu.add
            )
            tn = small_pool.tile([ROWS, K], dt.float32)
            nc.vector.tensor_scalar(tn[:], tf[:], -1.0, 1.0, op0=alu.mult, op1=alu.add)
            cneg = small_pool.tile([ROWS, K], dt.float32)
            nc.vector.tensor_tensor_scan(
                cneg[:], tn[:], zeros[:], 0.0, op0=alu.add, op1=alu.add
            )

            # masks: in-class AND within the first num_{pos,neg} of that class
            maskp = small_pool.tile([ROWS, K], dt.float32)
            nc.vector.scalar_tensor_tensor(
                maskp[:], cpos[:], float(num_pos), tf[:], op0=alu.is_le, op1=alu.mult
            )
            maskn = small_pool.tile([ROWS, K], dt.float32)
            nc.vector.scalar_tensor_tensor(
                maskn[:], cneg[:], float(num_neg), tn[:], op0=alu.is_le, op1=alu.mult
            )

            outsb = small_pool.tile([ROWS, 3], dt.float32)
            junk0 = small_pool.tile([ROWS, K], dt.float32)
            nc.vector.scalar_tensor_tensor(
                junk0[:], ce[:], 1.0, maskp[:],
                op0=alu.mult, op1=alu.mult, accum_out=outsb[:, 1:2],
            )
            junk1 = small_pool.tile([ROWS, K], dt.float32)
            nc.vector.scalar_tensor_tensor(
                junk1[:], ce[:], 1.0, maskn[:],
                op0=alu.mult, op1=alu.mult, accum_out=outsb[:, 2:3],
            )

            # ---- phase A compute: per-row positive counts ----
            # Row sums split across DVE (reduce_sum) and ACT (activation
            # accumulate) so neither engine extends past the DMA stream.
            row0 = 0
            for g, sz in enumerate(GROUPS):
                for r in range(sz):
                    col = statsf[:, row0 + r: row0 + r + 1]
                    if r % 2 == 0:
                        nc.vector.reduce_sum(
                            out=col, in_=bigts[g][:, r, :],
                            axis=mybir.AxisListType.X,
                        )
                    else:
                        scratch = big_pool.tile(
                            [128, F], dt.float32, tag=f"scratch{g % 2}"
                        )
                        nc.scalar.activation(
                            scratch[:], bigts[g][:, r, :], af.Copy, accum_out=col
                        )
                row0 += sz

            cnt_psum = psum_pool.tile([ROWS, 1], dt.float32)
            # counts[b] = sum_p statsf[p, b]
            nc.tensor.matmul(cnt_psum[:], statsf[:], ones[:], start=True, stop=True)
            nc.scalar.copy(outsb[:, 0:1], cnt_psum[:])

            nc.sync.dma_start(out.ap(), outsb[:])

    nc.compile()
    _cache[key] = nc
    return nc


def _host_exact(inputs, target, num_pos, num_neg):
    """Exact replication of the reference (jax on CPU). Safety fallback only."""
    import jax
    import jax.numpy as jnp

    cpu = jax.devices("cpu")[0]
    with jax.default_device(cpu):
        inputs = jnp.asarray(inputs)
        target = jnp.asarray(target)
        scores = jax.random.uniform(jax.random.key(42), (B, N))
        is_pos = target == 1
        is_neg = target == 0
        count_pos = is_pos.sum(axis=-1)
        min_pos = jnp.minimum(count_pos, num_pos)
        min_neg = jnp.minimum((count_pos * num_neg) // num_pos, num_neg)
        logp = jax.nn.log_softmax(inputs, axis=-1)
        ce = -jnp.take_along_axis(logp, target[..., None], axis=-1)[..., 0]

        def sampled_mean(mask, k, min_k):
            s = jnp.where(mask, scores, -jnp.inf)
            _, idx = jax.lax.top_k(s, k)
            sel = jnp.take_along_axis(ce, idx, axis=-1)
            valid = jnp.arange(k)[None, :] < min_k[:, None]
            return jnp.where(valid, sel, 0.0).sum(axis=-1) / jnp.maximum(min_k, 1)

        pos_loss = sampled_mean(is_pos, num_pos, min_pos)
        neg_loss = sampled_mean(is_neg, num_neg, min_neg)
        res = ((pos_loss + neg_loss) * 0.5).mean()
    return np.asarray(jax.device_get(res)).astype(np.float32)


def kernel(**inputs) -> np.ndarray:
    from concourse.bass_utils import run_bass_kernel_spmd

    x = np.ascontiguousarray(np.asarray(inputs["inputs"], dtype=np.float32))
    target = np.ascontiguousarray(np.asarray(inputs["target"], dtype=np.int32))
    num_pos = int(np.asarray(inputs["num_pos"]))
    num_neg = int(np.asarray(inputs["num_neg"]))

    if num_pos <= 0 or num_pos > K or num_neg < 0 or num_neg > K:
        # degenerate configs the device program doesn't cover
        return _host_exact(x, target, num_pos, num_neg)

    perm = _perm()
    gt = np.ascontiguousarray(np.take_along_axis(target, perm, axis=1))
    gx0 = np.ascontiguousarray(np.take_along_axis(x[:, :, 0], perm, axis=1))
    gx1 = np.ascontiguousarray(np.take_along_axis(x[:, :, 1], perm, axis=1))

    nc = _build_nc(num_pos, num_neg)
    core_ids = list(range(NCORES))
    in_maps = [
        {
            "tgt": target[c * ROWS:(c + 1) * ROWS],
            "gt": gt[c * ROWS:(c + 1) * ROWS],
            "gx0": gx0[c * ROWS:(c + 1) * ROWS],
            "gx1": gx1[c * ROWS:(c + 1) * ROWS],
        }
        for c in core_ids
    ]
    res = run_bass_kernel_spmd(nc, in_maps, core_ids, trace=_cache.get("trace", False))
    _cache["last_res"] = res
    outs = np.concatenate([res.results[c]["out"] for c in core_ids], axis=0)  # [B,3]

    count = np.rint(outs[:, 0]).astype(np.int64)
    s_pos = outs[:, 1].astype(np.float32)
    s_neg = outs[:, 2].astype(np.float32)

    min_pos = np.minimum(count, num_pos)                          # exact int
    min_neg = np.minimum((count * num_neg) // num_pos, num_neg)   # exact int

    # Guard: the K-prefix must contain every selected sample; count_neg must
    # cover min_neg (else reference semantics touch -inf slots).  Never fires
    # for the target data (binomial tails ~1e-80); fallback stays exact.
    prefix_pos = gt.sum(axis=1, dtype=np.int64)
    prefix_neg = K - prefix_pos
    count_neg = N - count
    if (
        (prefix_pos < min_pos).any()
        or (prefix_neg < min_neg).any()
        or (count_neg < min_neg).any()
    ):
        return _host_exact(x, target, num_pos, num_neg)

    pos_loss = s_pos / np.maximum(min_pos, 1).astype(np.float32)
    neg_loss = s_neg / np.maximum(min_neg, 1).astype(np.float32)
    loss = np.float32(0.5) * (pos_loss + neg_loss)
    return np.asarray(loss.mean(), dtype=np.float32)


# revision 27
# speedup vs baseline: 1.1047x; 1.0129x over previous
"""Balanced CE loss kernel for Trainium2 (8 NeuronCores, data parallel).

Math recap of the reference:
  - ce[b,n] = -log_softmax(inputs[b,n,:2])[target[b,n]]
            = softplus((x0-x1) * (2*t-1))           (two-class CE)
  - scores = uniform(key(42), (B,N))  -- a COMPILE-TIME CONSTANT
  - per row: mean of ce over the top-`num_pos`-by-score positives and the
    top-`num_neg`-by-score negatives; valid-count capped by count_pos.
  - loss = mean_b 0.5 * (pos_mean + neg_mean)

Only ~64 data-dependent positions per row matter, all of which lie inside the
first K=512 positions of the (constant) score-descending order with
overwhelming probability (Binomial(512, 1/2) tails; also checked at runtime
with an exact fallback).  So per core the device:
  1. streams its 16-row shard of `target` (the memory-bound bulk) and
     computes per-row positive counts (DVE reduce + PE partition-reduce),
  2. computes ce on the 512-long score-ordered prefix (gathered with
     constant indices), selects the first num_pos positives / num_neg
     negatives via a hardware prefix scan, and reduces to per-row sums.
Host does the final 128-row scalar arithmetic with exact integer math.
"""

import numpy as np

B, N, C = 128, 131072, 2
NCORES = 8
ROWS = B // NCORES  # 16 rows per core
K = 512             # score-order prefix depth per row
F = N // 128        # 1024 free elements per partition per row

_cache = {}


def _perm():
    """[B, K] int64: first K positions of each row in score-descending order.

    Must match jax.lax.top_k tie-breaking on the reference's scores exactly,
    so compute it with jax.lax.top_k on the very same scores (CPU backend;
    threefry PRNG is backend-deterministic).
    """
    if "perm" not in _cache:
        import jax

        cpu = jax.devices("cpu")[0]
        with jax.default_device(cpu):
            scores = jax.random.uniform(jax.random.key(42), (B, N), dtype=jax.numpy.float32)
            _, idx = jax.lax.top_k(scores, K)
        _cache["perm"] = np.asarray(jax.device_get(idx)).astype(np.int64)
    return _cache["perm"]


def _build_nc(num_pos: int, num_neg: int):
    """Compile the single-core Bass program (same NEFF on all 8 cores)."""
    key = ("nc", num_pos, num_neg)
    if key in _cache:
        return _cache[key]

    import concourse.bacc as bacc
    import concourse.bass as bass
    import concourse.mybir as mybir
    import concourse.tile as tile

    dt = mybir.dt
    af = mybir.ActivationFunctionType
    alu = mybir.AluOpType

    nc = bacc.Bacc("TRN2", target_bir_lowering=False, debug=False)

    tgt = nc.dram_tensor("tgt", [ROWS, N], dt.int32, kind="ExternalInput")
    gt = nc.dram_tensor("gt", [ROWS, K], dt.int32, kind="ExternalInput")
    gx0 = nc.dram_tensor("gx0", [ROWS, K], dt.float32, kind="ExternalInput")
    gx1 = nc.dram_tensor("gx1", [ROWS, K], dt.float32, kind="ExternalInput")
    out = nc.dram_tensor("out", [ROWS, 3], dt.float32, kind="ExternalOutput")

    with tile.TileContext(nc) as tc:
        with (
            tc.tile_pool(name="big", bufs=1) as big_pool,
            tc.tile_pool(name="stats", bufs=1) as stats_pool,
            tc.tile_pool(name="small", bufs=1) as small_pool,
            tc.tile_pool(name="psum", bufs=1, space=bass.MemorySpace.PSUM) as psum_pool,
        ):
            # Small prefix DMAs go on the gpsimd (SWDGE) queue so the sync
            # (HWDGE) queue starts streaming the big target DMAs with no
            # delay.  The prefix compute chain sits early in program order
            # so Tile runs it on DVE/ACT while the big DMAs stream; the
            # count reduces are split DVE/ACT so neither extends past the
            # DMA window.
            gtt = small_pool.tile([ROWS, K], dt.int32)
            x0 = small_pool.tile([ROWS, K], dt.float32)
            x1 = small_pool.tile([ROWS, K], dt.float32)
            nc.gpsimd.dma_start(gtt[:], gt.ap())
            nc.gpsimd.dma_start(x0[:], gx0.ap())
            nc.gpsimd.dma_start(x1[:], gx1.ap())

            statsf = stats_pool.tile([128, ROWS], dt.float32)
            ones = stats_pool.tile([128, 1], dt.float32)
            nc.vector.memset(ones[:], 1.0)

            # 2MB tiles keep the HWDGE queue saturated; the last two groups
            # are half-size so the final row-sums tail past the DMA stream
            # is minimal.
            GROUPS = [4, 4, 4, 2, 1, 1]
            assert sum(GROUPS) == ROWS
            tgt_ap = tgt.ap()
            bigts = []
            row0 = 0
            for g, sz in enumerate(GROUPS):
                bigt = big_pool.tile([128, sz, F], dt.int32, tag=f"big{g}")
                src = tgt_ap[row0:row0 + sz, :].rearrange("r (p f) -> p r f", p=128)
                nc.sync.dma_start(bigt[:], src)
                bigts.append(bigt)
                row0 += sz

            # ---- phase B: prefix selection + CE on [ROWS, K] ----
            tf = small_pool.tile([ROWS, K], dt.float32)
            nc.vector.tensor_copy(tf[:], gtt[:])  # i32 -> f32

            d = small_pool.tile([ROWS, K], dt.float32)
            nc.vector.tensor_sub(d[:], x0[:], x1[:])
            sgn = small_pool.tile([ROWS, K], dt.float32)
            nc.vector.tensor_scalar(sgn[:], tf[:], 2.0, -1.0, op0=alu.mult, op1=alu.add)
            dd = small_pool.tile([ROWS, K], dt.float32)
            nc.vector.tensor_mul(dd[:], d[:], sgn[:])
            # ce = softplus(dd) = relu(dd) + ln(1 + exp(-|dd|))
            # relu/abs on DVE; only exp/ln need the ACT tables
            rl = small_pool.tile([ROWS, K], dt.float32)
            nc.vector.tensor_scalar_max(rl[:], dd[:], 0.0)
            ab = small_pool.tile([ROWS, K], dt.float32)
            nc.vector.scalar_tensor_tensor(
                ab[:], rl[:], 2.0, dd[:], op0=alu.mult, op1=alu.subtract
            )
            ex = small_pool.tile([ROWS, K], dt.float32)
            nc.scalar.activation(ex[:], ab[:], af.Exp, scale=-1.0)
            ln = small_pool.tile([ROWS, K], dt.float32)
            nc.scalar.activation(ln[:], ex[:], af.Ln, bias=1.0)
            ce = small_pool.tile([ROWS, K], dt.float32)
            nc.vector.tensor_add(ce[:], rl[:], ln[:])

            # cumulative counts along the score order (gpsimd: it is idle,
            # and this chain only needs tf -- fully parallel to the DVE ce
            # chain)
            zeros = small_pool.tile([ROWS, K], dt.float32)
            nc.gpsimd.memset(zeros[:], 0.0)
            cpos = small_pool.tile([ROWS, K], dt.float32)
            nc.vector.tensor_tensor_scan(
                cpos[:], tf[:], zeros[:], 0.0, op0=alu.add, op1=alu.add
            )
            tn = small_pool.tile([ROWS, K], dt.float32)
            nc.vector.tensor_scalar(tn[:], tf[:], -1.0, 1.0, op0=alu.mult, op1=alu.add)
            cneg = small_pool.tile([ROWS, K], dt.float32)
            nc.vector.tensor_tensor_scan(
                cneg[:], tn[:], zeros[:], 0.0, op0=alu.add, op1=alu.add
            )

            # masks: in-class AND within the first num_{pos,neg} of that class
            maskp = small_pool.tile([ROWS, K], dt.float32)
            nc.vector.scalar_tensor_tensor(
                maskp[:], cpos[:], float(num_pos), tf[:], op0=alu.is_le, op1=alu.mult
            )
            maskn = small_pool.tile([ROWS, K], dt.float32)
            nc.vector.scalar_tensor_tensor(
                maskn[:], cneg[:], float(num_neg), tn[:], op0=alu.is_le, op1=alu.mult
            )

            outsb = small_pool.tile([ROWS, 3], dt.float32)
            junk0 = small_pool.tile([ROWS, K], dt.float32)
            nc.vector.scalar_tensor_tensor(
                junk0[:], ce[:], 1.0, maskp[:],
                op0=alu.mult, op1=alu.mult, accum_out=outsb[:, 1:2],
            )
            junk1 = small_pool.tile([ROWS, K], dt.float32)
            nc.vector.scalar_tensor_tensor(
                junk1[:], ce[:], 1.0, maskn[:],
                op0=alu.mult, op1=alu.mult, accum_out=outsb[:, 2:3],
            )

            # ---- phase A compute: per-row positive counts ----
            # Row sums split across DVE (reduce_sum) and ACT (activation
            # accumulate) so neither engine extends past the DMA stream.
            row0 = 0
            for g, sz in enumerate(GROUPS):
                for r in range(sz):
                    col = statsf[:, row0 + r: row0 + r + 1]
                    if (row0 + r) % 2 == 0:
                        nc.vector.reduce_sum(
                            out=col, in_=bigts[g][:, r, :],
                            axis=mybir.AxisListType.X,
                        )
                    else:
                        scratch = big_pool.tile(
                            [128, F], dt.float32, tag=f"scratch{g % 2}"
                        )
                        nc.scalar.activation(
                            scratch[:], bigts[g][:, r, :], af.Copy, accum_out=col
                        )
                row0 += sz

            cnt_psum = psum_pool.tile([ROWS, 1], dt.float32)
            # counts[b] = sum_p statsf[p, b]
            nc.tensor.matmul(cnt_psum[:], statsf[:], ones[:], start=True, stop=True)
            nc.scalar.copy(outsb[:, 0:1], cnt_psum[:])

            nc.sync.dma_start(out.ap(), outsb[:])

    nc.compile()
    _cache[key] = nc
    return nc


def _host_exact(inputs, target, num_pos, num_neg):
    """Exact replication of the reference (jax on CPU). Safety fallback only."""
    import jax
    import jax.numpy as jnp

    cpu = jax.devices("cpu")[0]
    with jax.default_device(cpu):
        inputs = jnp.asarray(inputs)
        target = jnp.asarray(target)
        scores = jax.random.uniform(jax.random.key(42), (B, N))
        is_pos = target == 1
        is_neg = target == 0
        count_pos = is_pos.sum(axis=-1)
        min_pos = jnp.minimum(count_pos, num_pos)
        min_neg = jnp.minimum((count_pos * num_neg) // num_pos, num_neg)
        logp = jax.nn.log_softmax(inputs, axis=-1)
        ce = -jnp.take_along_axis(logp, target[..., None], axis=-1)[..., 0]

        def sampled_mean(mask, k, min_k):
            s = jnp.where(mask, scores, -jnp.inf)
            _, idx = jax.lax.top_k(s, k)
            sel = jnp.take_along_axis(ce, idx, axis=-1)
            valid = jnp.arange(k)[None, :] < min_k[:, None]
            return jnp.where(valid, sel, 0.0).sum(axis=-1) / jnp.maximum(min_k, 1)

        pos_loss = sampled_mean(is_pos, num_pos, min_pos)
        neg_loss = sampled_mean(is_neg, num_neg, min_neg)
        res = ((pos_loss + neg_loss) * 0.5).mean()
    return np.asarray(jax.device_get(res)).astype(np.float32)


def kernel(**inputs) -> np.ndarray:
    from concourse.bass_utils import run_bass_kernel_spmd

    x = np.ascontiguousarray(np.asarray(inputs["inputs"], dtype=np.float32))
    target = np.ascontiguousarray(np.asarray(inputs["target"], dtype=np.int32))
    num_pos = int(np.asarray(inputs["num_pos"]))
    num_neg = int(np.asarray(inputs["num_neg"]))

    if num_pos <= 0 or num_pos > K or num_neg < 0 or num_neg > K:
        # degenerate configs the device program doesn't cover
        return _host_exact(x, target, num_pos, num_neg)

    perm = _perm()
    gt = np.ascontiguousarray(np.take_along_axis(target, perm, axis=1))
    gx0 = np.ascontiguousarray(np.take_along_axis(x[:, :, 0], perm, axis=1))
    gx1 = np.ascontiguousarray(np.take_along_axis(x[:, :, 1], perm, axis=1))

    nc = _build_nc(num_pos, num_neg)
    core_ids = list(range(NCORES))
    in_maps = [
        {
            "tgt": target[c * ROWS:(c + 1) * ROWS],
            "gt": gt[c * ROWS:(c + 1) * ROWS],
            "gx0": gx0[c * ROWS:(c + 1) * ROWS],
            "gx1": gx1[c * ROWS:(c + 1) * ROWS],
        }
        for c in core_ids
    ]
    res = run_bass_kernel_spmd(nc, in_maps, core_ids, trace=_cache.get("trace", False))
    _cache["last_res"] = res
    outs = np.concatenate([res.results[c]["out"] for c in core_ids], axis=0)  # [B,3]

    count = np.rint(outs[:, 0]).astype(np.int64)
    s_pos = outs[:, 1].astype(np.float32)
    s_neg = outs[:, 2].astype(np.float32)

    min_pos = np.minimum(count, num_pos)                          # exact int
    min_neg = np.minimum((count * num_neg) // num_pos, num_neg)   # exact int

    # Guard: the K-prefix must contain every selected sample; count_neg must
    # cover min_neg (else reference semantics touch -inf slots).  Never fires
    # for the target data (binomial tails ~1e-80); fallback stays exact.
    prefix_pos = gt.sum(axis=1, dtype=np.int64)
    prefix_neg = K - prefix_pos
    count_neg = N - count
    if (
        (prefix_pos < min_pos).any()
        or (prefix_neg < min_neg).any()
        or (count_neg < min_neg).any()
    ):
        return _host_exact(x, target, num_pos, num_neg)

    pos_loss = s_pos / np.maximum(min_pos, 1).astype(np.float32)
    neg_loss = s_neg / np.maximum(min_neg, 1).astype(np.float32)
    loss = np.float32(0.5) * (pos_loss + neg_loss)
    return np.asarray(loss.mean(), dtype=np.float32)


# revision 29
# speedup vs baseline: 2.0760x; 1.8793x over previous
"""Balanced CE loss kernel for Trainium2 (8 NeuronCores, data parallel).

Math recap of the reference:
  - ce[b,n] = -log_softmax(inputs[b,n,:2])[target[b,n]]
            = softplus((x0-x1) * (2*t-1))           (two-class CE)
  - scores = uniform(key(42), (B,N))  -- a COMPILE-TIME CONSTANT
  - per row: mean of ce over the top-`num_pos`-by-score positives and the
    top-`num_neg`-by-score negatives; valid-count capped by count_pos.
  - loss = mean_b 0.5 * (pos_mean + neg_mean)

Key reductions:
  1. Only positions among each row's top-K (K=256) constant score order can
     be selected, so only those positions of inputs/target matter.
  2. count_pos only enters via min(count_pos, num_pos) and
     min((count_pos*num_neg)//num_pos, num_neg).  If the K-prefix already
     holds >= num_pos positives and >= num_neg negatives (checked EXACTLY on
     the host from the gathered prefix; bit-exact fallback otherwise), both
     saturate to num_pos / num_neg and the full count is never needed.

So each core only computes, for its 16 rows: ce over the K-prefix, a
hardware prefix-scan selection of the first num_pos positives / num_neg
negatives, and the two masked row sums.  The host does the constant
score-order gather and the final 128-row scalar math.
"""

import numpy as np

B, N, C = 128, 131072, 2
NCORES = 8
ROWS = B // NCORES  # 16 rows per core
K = 256             # score-order prefix depth per row

_cache = {}


def _perm():
    """[B, K] int64: first K positions of each row in score-descending order.

    Must match jax.lax.top_k tie-breaking on the reference's scores exactly,
    so compute it with jax.lax.top_k on the very same scores (CPU backend;
    threefry PRNG is backend-deterministic).
    """
    if "perm" not in _cache:
        import jax

        cpu = jax.devices("cpu")[0]
        with jax.default_device(cpu):
            scores = jax.random.uniform(jax.random.key(42), (B, N), dtype=jax.numpy.float32)
            _, idx = jax.lax.top_k(scores, K)
        _cache["perm"] = np.asarray(jax.device_get(idx)).astype(np.int64)
    return _cache["perm"]


def _build_nc(num_pos: int, num_neg: int):
    """Compile the single-core Bass program (same NEFF on all 8 cores)."""
    key = ("nc", num_pos, num_neg)
    if key in _cache:
        return _cache[key]

    import concourse.bacc as bacc
    import concourse.bass as bass
    import concourse.mybir as mybir
    import concourse.tile as tile

    dt = mybir.dt
    af = mybir.ActivationFunctionType
    alu = mybir.AluOpType

    nc = bacc.Bacc("TRN2", target_bir_lowering=False, debug=False)

    # pk packs [t_as_f32 | x0 | x1] along the free dim -> one input DMA
    pk = nc.dram_tensor("pk", [ROWS, 3 * K], dt.float32, kind="ExternalInput")
    out = nc.dram_tensor("out", [ROWS, 2], dt.float32, kind="ExternalOutput")

    with tile.TileContext(nc) as tc:
        with tc.tile_pool(name="small", bufs=1) as sp:
            # Warm the ACT Exp/Ln tables on dummy data so both table loads
            # (~1.3us each) run before the input DMA lands.
            warm = sp.tile([1, 2], dt.float32)
            nc.gpsimd.memset(warm[:], 0.5)
            warm2 = sp.tile([1, 2], dt.float32)
            nc.scalar.activation(warm2[:], warm[:], af.Exp)
            nc.scalar.activation(warm2[:], warm[:], af.Ln, bias=1.0)

            zeros = sp.tile([ROWS, K], dt.float32)
            nc.gpsimd.memset(zeros[:], 0.0)

            pkt = sp.tile([ROWS, 3 * K], dt.float32)
            nc.sync.dma_start(pkt[:], pk.ap())
            tf = pkt[:, 0:K]
            x0 = pkt[:, K:2 * K]
            x1 = pkt[:, 2 * K:3 * K]

            # ce chain (DVE feeds ACT as early as possible)
            d = sp.tile([ROWS, K], dt.float32)
            nc.vector.tensor_sub(d[:], x0, x1)
            sgn = sp.tile([ROWS, K], dt.float32)
            nc.vector.tensor_scalar(sgn[:], tf, 2.0, -1.0, op0=alu.mult, op1=alu.add)
            dd = sp.tile([ROWS, K], dt.float32)
            nc.vector.tensor_mul(dd[:], d[:], sgn[:])
            # softplus(dd) = relu(dd) + ln(1 + exp(-|dd|)), |dd| = 2*relu-dd
            rl = sp.tile([ROWS, K], dt.float32)
            nc.vector.tensor_scalar_max(rl[:], dd[:], 0.0)
            ab = sp.tile([ROWS, K], dt.float32)
            nc.vector.scalar_tensor_tensor(
                ab[:], rl[:], 2.0, dd[:], op0=alu.mult, op1=alu.subtract
            )
            ex = sp.tile([ROWS, K], dt.float32)
            nc.scalar.activation(ex[:], ab[:], af.Exp, scale=-1.0)
            ln = sp.tile([ROWS, K], dt.float32)
            nc.scalar.activation(ln[:], ex[:], af.Ln, bias=1.0)

            # selection masks (need only tf -- run on DVE while ACT works)
            tn = sp.tile([ROWS, K], dt.float32)
            nc.vector.tensor_scalar(tn[:], tf, -1.0, 1.0, op0=alu.mult, op1=alu.add)
            cpos = sp.tile([ROWS, K], dt.float32)
            nc.vector.tensor_tensor_scan(
                cpos[:], tf, zeros[:], 0.0, op0=alu.add, op1=alu.add
            )
            cneg = sp.tile([ROWS, K], dt.float32)
            nc.vector.tensor_tensor_scan(
                cneg[:], tn[:], zeros[:], 0.0, op0=alu.add, op1=alu.add
            )
            maskp = sp.tile([ROWS, K], dt.float32)
            nc.vector.scalar_tensor_tensor(
                maskp[:], cpos[:], float(num_pos), tf, op0=alu.is_le, op1=alu.mult
            )
            maskn = sp.tile([ROWS, K], dt.float32)
            nc.vector.scalar_tensor_tensor(
                maskn[:], cneg[:], float(num_neg), tn[:], op0=alu.is_le, op1=alu.mult
            )

            ce = sp.tile([ROWS, K], dt.float32)
            nc.vector.tensor_add(ce[:], rl[:], ln[:])

            outsb = sp.tile([ROWS, 2], dt.float32)
            junk0 = sp.tile([ROWS, K], dt.float32)
            nc.vector.scalar_tensor_tensor(
                junk0[:], ce[:], 1.0, maskp[:],
                op0=alu.mult, op1=alu.mult, accum_out=outsb[:, 0:1],
            )
            junk1 = sp.tile([ROWS, K], dt.float32)
            nc.vector.scalar_tensor_tensor(
                junk1[:], ce[:], 1.0, maskn[:],
                op0=alu.mult, op1=alu.mult, accum_out=outsb[:, 1:2],
            )

            nc.sync.dma_start(out.ap(), outsb[:])

    nc.compile()
    _cache[key] = nc
    return nc


def _host_exact(inputs, target, num_pos, num_neg):
    """Exact replication of the reference (jax on CPU). Safety fallback only."""
    import jax
    import jax.numpy as jnp

    cpu = jax.devices("cpu")[0]
    with jax.default_device(cpu):
        inputs = jnp.asarray(inputs)
        target = jnp.asarray(target)
        scores = jax.random.uniform(jax.random.key(42), (B, N))
        is_pos = target == 1
        is_neg = target == 0
        count_pos = is_pos.sum(axis=-1)
        min_pos = jnp.minimum(count_pos, num_pos)
        min_neg = jnp.minimum((count_pos * num_neg) // num_pos, num_neg)
        logp = jax.nn.log_softmax(inputs, axis=-1)
        ce = -jnp.take_along_axis(logp, target[..., None], axis=-1)[..., 0]

        def sampled_mean(mask, k, min_k):
            s = jnp.where(mask, scores, -jnp.inf)
            _, idx = jax.lax.top_k(s, k)
            sel = jnp.take_along_axis(ce, idx, axis=-1)
            valid = jnp.arange(k)[None, :] < min_k[:, None]
            return jnp.where(valid, sel, 0.0).sum(axis=-1) / jnp.maximum(min_k, 1)

        pos_loss = sampled_mean(is_pos, num_pos, min_pos)
        neg_loss = sampled_mean(is_neg, num_neg, min_neg)
        res = ((pos_loss + neg_loss) * 0.5).mean()
    return np.asarray(jax.device_get(res)).astype(np.float32)


def kernel(**inputs) -> np.ndarray:
    from concourse.bass_utils import run_bass_kernel_spmd

    x = np.ascontiguousarray(np.asarray(inputs["inputs"], dtype=np.float32))
    target = np.ascontiguousarray(np.asarray(inputs["target"], dtype=np.int32))
    num_pos = int(np.asarray(inputs["num_pos"]))
    num_neg = int(np.asarray(inputs["num_neg"]))

    if num_pos <= 0 or num_pos > K or num_neg < 0 or num_neg > K:
        # degenerate configs the device program doesn't cover
        return _host_exact(x, target, num_pos, num_neg)

    perm = _perm()
    gt = np.take_along_axis(target, perm, axis=1)          # [B, K] int32
    # Guard: with >= num_pos positives and >= num_neg negatives inside every
    # row's K-prefix, min_pos == num_pos and min_neg == num_neg exactly
    # ((c*nn)//np >= nn  <=>  c >= np for nn > 0), the selected samples all
    # lie inside the prefix, and count_pos is never needed.  Fall back to
    # the exact host computation otherwise (never fires for this data:
    # binomial(256, 1/2) tails; real-data margins are >= 100 of each).
    prefix_pos = gt.sum(axis=1, dtype=np.int64)
    prefix_neg = K - prefix_pos
    if (prefix_pos < num_pos).any() or (prefix_neg < num_neg).any():
        return _host_exact(x, target, num_pos, num_neg)

    pk = np.empty((B, 3 * K), dtype=np.float32)
    pk[:, 0:K] = gt
    pk[:, K:2 * K] = np.take_along_axis(x[:, :, 0], perm, axis=1)
    pk[:, 2 * K:3 * K] = np.take_along_axis(x[:, :, 1], perm, axis=1)

    nc = _build_nc(num_pos, num_neg)
    core_ids = list(range(NCORES))
    in_maps = [
        {"pk": np.ascontiguousarray(pk[c * ROWS:(c + 1) * ROWS])}
        for c in core_ids
    ]
    res = run_bass_kernel_spmd(nc, in_maps, core_ids, trace=_cache.get("trace", False))
    _cache["last_res"] = res
    outs = np.concatenate([res.results[c]["out"] for c in core_ids], axis=0)  # [B,2]

    pos_loss = outs[:, 0].astype(np.float32) / np.float32(num_pos)
    neg_loss = outs[:, 1].astype(np.float32) / np.float32(max(num_neg, 1))
    loss = np.float32(0.5) * (pos_loss + neg_loss)
    return np.asarray(loss.mean(), dtype=np.float32)


# revision 31
# speedup vs baseline: 2.2224x; 1.0705x over previous
"""Balanced CE loss kernel for Trainium2 (8 NeuronCores, data parallel).

Math recap of the reference:
  - ce[b,n] = -log_softmax(inputs[b,n,:2])[target[b,n]]
            = softplus((x0-x1) * (2*t-1))           (two-class CE)
  - scores = uniform(key(42), (B,N))  -- a COMPILE-TIME CONSTANT
  - per row: mean of ce over the top-`num_pos`-by-score positives and the
    top-`num_neg`-by-score negatives; valid-count capped by count_pos.
  - loss = mean_b 0.5 * (pos_mean + neg_mean)

Key reductions:
  1. Only positions among each row's top-K (K=256) constant score order can
     be selected, so only those positions of inputs/target matter.
  2. count_pos only enters via min(count_pos, num_pos) and
     min((count_pos*num_neg)//num_pos, num_neg).  If the K-prefix already
     holds >= num_pos positives and >= num_neg negatives (checked EXACTLY on
     the host from the gathered prefix; bit-exact fallback otherwise), both
     saturate to num_pos / num_neg and the full count is never needed.

So each core only computes, for its 16 rows: ce over the K-prefix, a
hardware prefix-scan selection of the first num_pos positives / num_neg
negatives, and the two masked row sums.  The host does the constant
score-order gather and the final 128-row scalar math.
"""

import numpy as np

B, N, C = 128, 131072, 2
NCORES = 8
ROWS = B // NCORES  # 16 rows per core
K = 256             # score-order prefix depth per row

_cache = {}


def _perm():
    """[B, K] int64: first K positions of each row in score-descending order.

    Must match jax.lax.top_k tie-breaking on the reference's scores exactly,
    so compute it with jax.lax.top_k on the very same scores (CPU backend;
    threefry PRNG is backend-deterministic).
    """
    if "perm" not in _cache:
        import jax

        cpu = jax.devices("cpu")[0]
        with jax.default_device(cpu):
            scores = jax.random.uniform(jax.random.key(42), (B, N), dtype=jax.numpy.float32)
            _, idx = jax.lax.top_k(scores, K)
        _cache["perm"] = np.asarray(jax.device_get(idx)).astype(np.int64)
    return _cache["perm"]


def _build_nc(num_pos: int, num_neg: int):
    """Compile the single-core Bass program (same NEFF on all 8 cores)."""
    key = ("nc", num_pos, num_neg)
    if key in _cache:
        return _cache[key]

    import concourse.bacc as bacc
    import concourse.bass as bass
    import concourse.mybir as mybir
    import concourse.tile as tile

    dt = mybir.dt
    af = mybir.ActivationFunctionType
    alu = mybir.AluOpType

    # Steer the ACT-table pass: by default it picks `exp_and_others` for Exp
    # and `natural_log` for Ln, which evict each other (1.28us reload on the
    # critical path).  Restrict Exp/Ln to the combined
    # `natural_log_exp_and_others` set (keeping every set's index intact so
    # act_func_set_id stays valid) -> a single table load serves both.
    if not _cache.get("act_tables_patched"):
        orig_get = bacc.get_activation_tables

        def _combined_tables(arch):
            tabs = orig_get(arch)
            combined = "natural_log_exp_and_others"
            if combined in tabs and {af.Exp, af.Ln} <= tabs[combined]:
                for name, fns in tabs.items():
                    if name != combined:
                        fns.discard(af.Exp)
                        fns.discard(af.Ln)
            return tabs

        bacc.get_activation_tables = _combined_tables
        _cache["act_tables_patched"] = True

    nc = bacc.Bacc("TRN2", target_bir_lowering=False, debug=False)

    # pk packs [t_as_f32 | x0 | x1] along the free dim -> one input DMA
    pk = nc.dram_tensor("pk", [ROWS, 3 * K], dt.float32, kind="ExternalInput")
    out = nc.dram_tensor("out", [ROWS, 2], dt.float32, kind="ExternalOutput")

    with tile.TileContext(nc) as tc:
        with tc.tile_pool(name="small", bufs=1) as sp:
            zeros = sp.tile([ROWS, K], dt.float32)
            nc.gpsimd.memset(zeros[:], 0.0)

            pkt = sp.tile([ROWS, 3 * K], dt.float32)
            nc.sync.dma_start(pkt[:], pk.ap())
            tf = pkt[:, 0:K]
            x0 = pkt[:, K:2 * K]
            x1 = pkt[:, 2 * K:3 * K]

            # ce chain (DVE feeds ACT as early as possible)
            d = sp.tile([ROWS, K], dt.float32)
            nc.vector.tensor_sub(d[:], x0, x1)
            sgn = sp.tile([ROWS, K], dt.float32)
            nc.vector.tensor_scalar(sgn[:], tf, 2.0, -1.0, op0=alu.mult, op1=alu.add)
            dd = sp.tile([ROWS, K], dt.float32)
            nc.vector.tensor_mul(dd[:], d[:], sgn[:])
            # softplus(dd) = relu(dd) + ln(1 + exp(-|dd|)), |dd| = 2*relu-dd
            rl = sp.tile([ROWS, K], dt.float32)
            nc.vector.tensor_scalar_max(rl[:], dd[:], 0.0)
            ab = sp.tile([ROWS, K], dt.float32)
            nc.vector.scalar_tensor_tensor(
                ab[:], rl[:], 2.0, dd[:], op0=alu.mult, op1=alu.subtract
            )
            ex = sp.tile([ROWS, K], dt.float32)
            nc.scalar.activation(ex[:], ab[:], af.Exp, scale=-1.0)
            ln = sp.tile([ROWS, K], dt.float32)
            nc.scalar.activation(ln[:], ex[:], af.Ln, bias=1.0)

            # selection masks (need only tf -- run on DVE while ACT works)
            tn = sp.tile([ROWS, K], dt.float32)
            nc.vector.tensor_scalar(tn[:], tf, -1.0, 1.0, op0=alu.mult, op1=alu.add)
            cpos = sp.tile([ROWS, K], dt.float32)
            nc.vector.tensor_tensor_scan(
                cpos[:], tf, zeros[:], 0.0, op0=alu.add, op1=alu.add
            )
            cneg = sp.tile([ROWS, K], dt.float32)
            nc.vector.tensor_tensor_scan(
                cneg[:], tn[:], zeros[:], 0.0, op0=alu.add, op1=alu.add
            )
            maskp = sp.tile([ROWS, K], dt.float32)
            nc.vector.scalar_tensor_tensor(
                maskp[:], cpos[:], float(num_pos), tf, op0=alu.is_le, op1=alu.mult
            )
            maskn = sp.tile([ROWS, K], dt.float32)
            nc.vector.scalar_tensor_tensor(
                maskn[:], cneg[:], float(num_neg), tn[:], op0=alu.is_le, op1=alu.mult
            )

            ce = sp.tile([ROWS, K], dt.float32)
            nc.vector.tensor_add(ce[:], rl[:], ln[:])

            outsb = sp.tile([ROWS, 2], dt.float32)
            junk0 = sp.tile([ROWS, K], dt.float32)
            nc.vector.scalar_tensor_tensor(
                junk0[:], ce[:], 1.0, maskp[:],
                op0=alu.mult, op1=alu.mult, accum_out=outsb[:, 0:1],
            )
            junk1 = sp.tile([ROWS, K], dt.float32)
            nc.vector.scalar_tensor_tensor(
                junk1[:], ce[:], 1.0, maskn[:],
                op0=alu.mult, op1=alu.mult, accum_out=outsb[:, 1:2],
            )

            nc.sync.dma_start(out.ap(), outsb[:])

    nc.compile()
    _cache[key] = nc
    return nc


def _host_exact(inputs, target, num_pos, num_neg):
    """Exact replication of the reference (jax on CPU). Safety fallback only."""
    import jax
    import jax.numpy as jnp

    cpu = jax.devices("cpu")[0]
    with jax.default_device(cpu):
        inputs = jnp.asarray(inputs)
        target = jnp.asarray(target)
        scores = jax.random.uniform(jax.random.key(42), (B, N))
        is_pos = target == 1
        is_neg = target == 0
        count_pos = is_pos.sum(axis=-1)
        min_pos = jnp.minimum(count_pos, num_pos)
        min_neg = jnp.minimum((count_pos * num_neg) // num_pos, num_neg)
        logp = jax.nn.log_softmax(inputs, axis=-1)
        ce = -jnp.take_along_axis(logp, target[..., None], axis=-1)[..., 0]

        def sampled_mean(mask, k, min_k):
            s = jnp.where(mask, scores, -jnp.inf)
            _, idx = jax.lax.top_k(s, k)
            sel = jnp.take_along_axis(ce, idx, axis=-1)
            valid = jnp.arange(k)[None, :] < min_k[:, None]
            return jnp.where(valid, sel, 0.0).sum(axis=-1) / jnp.maximum(min_k, 1)

        pos_loss = sampled_mean(is_pos, num_pos, min_pos)
        neg_loss = sampled_mean(is_neg, num_neg, min_neg)
        res = ((pos_loss + neg_loss) * 0.5).mean()
    return np.asarray(jax.device_get(res)).astype(np.float32)


def kernel(**inputs) -> np.ndarray:
    from concourse.bass_utils import run_bass_kernel_spmd

    x = np.ascontiguousarray(np.asarray(inputs["inputs"], dtype=np.float32))
    target = np.ascontiguousarray(np.asarray(inputs["target"], dtype=np.int32))
    num_pos = int(np.asarray(inputs["num_pos"]))
    num_neg = int(np.asarray(inputs["num_neg"]))

    if num_pos <= 0 or num_pos > K or num_neg < 0 or num_neg > K:
        # degenerate configs the device program doesn't cover
        return _host_exact(x, target, num_pos, num_neg)

    perm = _perm()
    gt = np.take_along_axis(target, perm, axis=1)          # [B, K] int32
    # Guard: with >= num_pos positives and >= num_neg negatives inside every
    # row's K-prefix, min_pos == num_pos and min_neg == num_neg exactly
    # ((c*nn)//np >= nn  <=>  c >= np for nn > 0), the selected samples all
    # lie inside the prefix, and count_pos is never needed.  Fall back to
    # the exact host computation otherwise (never fires for this data:
    # binomial(256, 1/2) tails; real-data margins are >= 100 of each).
    prefix_pos = gt.sum(axis=1, dtype=np.int64)
    prefix_neg = K - prefix_pos
    if (prefix_pos < num_pos).any() or (prefix_neg < num_neg).any():
        return _host_exact(x, target, num_pos, num_neg)

    pk = np.empty((B, 3 * K), dtype=np.float32)
    pk[:, 0:K] = gt
    pk[:, K:2 * K] = np.take_along_axis(x[:, :, 0], perm, axis=1)
    pk[:, 2 * K:3 * K] = np.take_along_axis(x[:, :, 1], perm, axis=1)

    nc = _build_nc(num_pos, num_neg)
    core_ids = list(range(NCORES))
    in_maps = [
        {"pk": np.ascontiguousarray(pk[c * ROWS:(c + 1) * ROWS])}
        for c in core_ids
    ]
    res = run_bass_kernel_spmd(nc, in_maps, core_ids, trace=_cache.get("trace", False))
    _cache["last_res"] = res
    outs = np.concatenate([res.results[c]["out"] for c in core_ids], axis=0)  # [B,2]

    pos_loss = outs[:, 0].astype(np.float32) / np.float32(num_pos)
    neg_loss = outs[:, 1].astype(np.float32) / np.float32(max(num_neg, 1))
    loss = np.float32(0.5) * (pos_loss + neg_loss)
    return np.asarray(loss.mean(), dtype=np.float32)
